# revision 3
# baseline (speedup 1.0000x reference)
"""Trainium2 Bass kernel for a 16-head dense attention layer (v2, bf16).

Problem: x[1,4096,1024] @ w_qkv[1024,3072] -> 16-head attention (N=4096,
D=64) -> @ w_out[1024,1024].

Sharding: tensor-parallel over heads across 8 NeuronCores (2 heads/core).
Each core computes q/k/v for its 2 heads (weights column-sliced on host),
attention with a fused, max-free softmax (scores are bounded so exp never
overflows in fp32; denominator comes from an appended ones-column in V),
then an AllToAll converts the head-sharded attention output into a
sequence-sharded layout so every core applies the full output projection
to its own 512 rows. Host concatenates the 8 row slices.

v2 vs v1: all matmul operands bf16 (PSUM accumulation stays fp32; rel-err
budget is 2e-2), V projected directly in [keys, dims] layout (no PE
transposes), one merged x DMA per chunk, lag-1 attention interleave in
phase 1, one AllToAll per stripe so only the last stripe's output
projection sits on the tail, and softmax normalization reads an SBUF copy
of the accumulator so PSUM banks free early at stripe boundaries.
"""

import os
import numpy as np

N_CORES = 8
N = 4096
HIDDEN = 1024
D = 64
HPC = 2  # heads per core
AD = HPC * D  # 128 att-dim rows per core
NT = N // 128  # 32 k-tiles of 128
HT = HIDDEN // 128  # 8 hidden tiles
QCHUNK = 1024
NQC = N // QCHUNK  # 4 q-chunks (stripes)
NSLICE = N // N_CORES  # 512 rows of output per core

_CACHE = {}


def _build(mm_mode: str = "bf16", skip_a2a: bool = False, att_nt: int = NT):
    import concourse.bass as bass
    import concourse.mybir as mybir
    import concourse.tile as tile
    from concourse import bacc

    DT = mybir.dt.float32
    DTM = mybir.dt.bfloat16

    AF = mybir.ActivationFunctionType

    nc = bacc.Bacc("TRN2", debug=False, num_devices=N_CORES)

    xT = nc.dram_tensor("xT", [HIDDEN, N], DTM, kind="ExternalInput").ap()
    wq = nc.dram_tensor("wq", [HIDDEN, AD], DTM, kind="ExternalInput").ap()
    wk = nc.dram_tensor("wk", [HIDDEN, AD], DTM, kind="ExternalInput").ap()
    wv = nc.dram_tensor("wv", [HIDDEN, AD], DTM, kind="ExternalInput").ap()
    bq = nc.dram_tensor("bq", [AD, 1], DT, kind="ExternalInput").ap()
    bk = nc.dram_tensor("bk", [AD, 1], DT, kind="ExternalInput").ap()
    bvT = nc.dram_tensor("bvT", [1, AD], DT, kind="ExternalInput").ap()
    wo = nc.dram_tensor("wo", [HIDDEN, HIDDEN], DTM, kind="ExternalInput").ap()
    bo = nc.dram_tensor("bo", [1, HIDDEN], DT, kind="ExternalInput").ap()
    out = nc.dram_tensor("out", [NSLICE, HIDDEN], DT, kind="ExternalOutput").ap()

    with tile.TileContext(nc) as tc:
        with (
            tc.tile_pool(name="sb", bufs=1) as sb,
            tc.tile_pool(name="ps", bufs=2, space="PSUM") as ps,
            tc.tile_pool(name="dram", bufs=1, space="DRAM") as dram,
        ):
            # Global reordering: the sequence axis n is processed in
            # "stripe" order n' = (m, j, t) <-> n = 512*j + 128*m + t
            # (m: stripe 0..3, j: destination core 0..7, t: 0..127).
            # Attention is permutation-invariant in the key axis as long as
            # k and v use the same order, and the q axis just needs the
            # inverse map applied at output -- which the AllToAll block
            # routing does implicitly. Stripe m's attention output IS the
            # m-th out-row-tile of every core, so each stripe's AllToAll +
            # out-projection pipeline behind the next stripe's attention.

            bvT_sb = sb.tile([1, AD], DT)
            # qkv weights: one DMA each, [1024, 128] folded to [128, 8*128]
            wq_sb = sb.tile([128, HT * AD], DTM)
            wk_sb = sb.tile([128, HT * AD], DTM)
            wv_sb = sb.tile([128, HT * AD], DTM)
            bq_sb = sb.tile([AD, 1], DT)
            bk_sb = sb.tile([AD, 1], DT)
            bv_bc = sb.tile([128, AD], DT)

            def emit_weight_loads():
                for w_sb, wsrc in ((wq_sb, wq), (wk_sb, wk), (wv_sb, wv)):
                    nc.sync.dma_start(
                        w_sb[:].rearrange("p (a c) -> p a c", a=HT),
                        wsrc.rearrange("(a p) c -> p a c", p=128),
                    )
                nc.sync.dma_start(bq_sb[:], bq[:])
                nc.sync.dma_start(bk_sb[:], bk[:])
                nc.sync.dma_start(bvT_sb[:], bvT[:])
                nc.gpsimd.partition_broadcast(bv_bc[:], bvT_sb[:1, :])

            def wslice(w_sb, i):
                return w_sb[:, i * AD : (i + 1) * AD]

            # Host pre-permutes x columns into stripe order n' = (m, j, t),
            # so streaming, qT, kTc, v_nat are all plain contiguous in n'.
            qT = sb.tile([AD, N], DTM)
            kTc = [sb.tile([AD, 512], DTM, name=f"kTc{c}", tag="kTc", bufs=HT) for c in range(HT)]
            att_m = [sb.tile([AD, QCHUNK], DTM, name=f"attm{m}", tag="attm", bufs=NQC) for m in range(NQC)]
            # v in natural [keys, dims] layout: per chunk [128, (j, h, D+1)],
            # ones column at slot D of each head for the softmax denominator.
            v_nat = [
                sb.tile([128, 4 * HPC * (D + 1)], DTM, name=f"vn{c}", tag="vnat", bufs=HT)
                for c in range(HT)
            ]
            wo_sb = [sb.tile([128, HIDDEN], DTM, name=f"wo{i}", tag="wo", bufs=HT) for i in range(HT)]
            bo_bc = sb.tile([128, HIDDEN], DT)

            a2a_in = [
                dram.tile([N_CORES, AD, 128], DTM, name=f"a2ai{m}", tag="a2ai", bufs=NQC)
                for m in range(NQC)
            ]
            a2a_out = [
                dram.tile([N_CORES, AD, 128], DTM, name=f"a2ao{m}", tag="a2ao", bufs=NQC)
                for m in range(NQC)
            ]

            def vn_h(c, j, h):
                """[128 keys, D+1] slice of chunk c's v for k-tile j, head h."""
                base = (j * HPC + h) * (D + 1)
                return v_nat[c][:, base : base + D + 1]

            # ---- emission helpers --------------------------------------
            # All non-attention PE work is emitted as small "pieces" (2-4
            # matmuls, ~0.4-0.9us) with DVE partial accumulation, woven
            # between attention (scores+exp+AV) pairs. A long uninterrupted
            # matmul block would stall the in-order PE stream past the ~2
            # tiles of exp backlog the s_ps double-buffer can hold, idling
            # the ACT engine (the overall bottleneck) by its own duration.
            # Pieces are always injected in PAIRS so the number of ps_big
            # allocations between consecutive s_ps allocations stays even
            # and s_ps keeps alternating between its two buffers.
            xts = []

            def emit_xt(cp, eng=None):
                """x chunk load via the (otherwise idle) Pool queue so the
                SP queue's weight DMAs never delay it; 8 bufs = fully
                prefetched, no reuse dependency between chunks (they also
                stay resident for the deferred q projections)."""
                cs = slice(cp * 512, (cp + 1) * 512)
                xt = sb.tile([128, HT * 512], DTM, name="xt", tag="xt", bufs=HT)
                xts.append(xt)
                (eng or nc.gpsimd).dma_start(
                    xt[:].rearrange("p (a t) -> p a t", a=HT),
                    xT[:, cs].rearrange("(a p) t -> p a t", p=128),
                )

            def qk_pieces(cp, w_sb, b_sb, dst, npiece):
                """q or k projection for chunk cp as npiece thunks."""
                per = HT // npiece
                thunks = []
                for pc in range(npiece):
                    def piece(pc=pc):
                        pp = ps.tile([128, 512], DT, name="pp", tag="ps_big")
                        for i in range(per * pc, per * (pc + 1)):
                            nc.tensor.matmul(
                                pp[:AD, :], wslice(w_sb, i),
                                xts[cp][:, i * 512 : (i + 1) * 512],
                                start=(i == per * pc), stop=(i == per * (pc + 1) - 1),
                            )
                        if pc == 0:
                            nc.vector.tensor_scalar_add(dst, pp[:AD, :], b_sb[:])
                        else:
                            nc.vector.tensor_add(dst, dst, pp[:AD, :])
                    thunks.append(piece)
                return thunks

            def k_pieces(cp):
                return qk_pieces(cp, wk_sb, bk_sb, kTc[cp][:], 2)

            def q_pieces(cp, npiece=2):
                cs = slice(cp * 512, (cp + 1) * 512)
                return qk_pieces(cp, wq_sb, bq_sb, qT[:, cs], npiece)

            def v_pieces(cp):
                """v directly in [keys, dims] layout: x-chunk tile as the
                stationary operand, wv moving; out partitions are the 128
                keys of k-tile j. Two thunks of two k-tiles each."""
                thunks = []
                for half in (0, 1):
                    def piece(half=half):
                        pv = ps.tile([128, 256], DT, name="pv", tag="ps_big")
                        for jj in (0, 1):
                            j = 2 * half + jj
                            for i in range(HT):
                                nc.tensor.matmul(
                                    pv[:, jj * 128 : (jj + 1) * 128],
                                    xts[cp][:, i * 512 + j * 128 : i * 512 + (j + 1) * 128],
                                    wslice(wv_sb, i),
                                    start=(i == 0), stop=(i == HT - 1),
                                )
                        vn4 = v_nat[cp][:].rearrange("p (j h x) -> p j h x", j=4, x=D + 1)
                        for jj in (0, 1):
                            j = 2 * half + jj
                            nc.vector.tensor_add(
                                vn4[:, j, :, :D],
                                pv[:, jj * 128 : (jj + 1) * 128].rearrange(
                                    "p (h d) -> p h d", h=HPC
                                ),
                                bv_bc[:].rearrange("p (h d) -> p h d", h=HPC),
                            )
                        nc.vector.memset(
                            vn4[:, 2 * half : 2 * half + 2, :, D : D + 1], 1.0
                        )
                    thunks.append(piece)
                return thunks

            def emit_att_kt(m, kt_i, h, accs):
                hs = slice(h * D, (h + 1) * D)
                s_ps = ps.tile([128, QCHUNK], DT, name="s_ps", tag="ps_big")
                for half in range(2):
                    nc.tensor.matmul(
                        s_ps[:, half * 512 : (half + 1) * 512],
                        kTc[kt_i // 4][hs, (kt_i % 4) * 128 : (kt_i % 4 + 1) * 128],
                        qT[hs, m * QCHUNK + half * 512 : m * QCHUNK + (half + 1) * 512],
                        start=True, stop=True,
                    )
                p_sb = sb.tile([128, QCHUNK], DTM, name="p_sb", tag="p_sb", bufs=3)
                nc.scalar.activation(p_sb[:], s_ps[:], AF.Exp, scale=0.125)
                for half in range(2):
                    hsl = slice(half * 512, (half + 1) * 512)
                    nc.tensor.matmul(
                        accs[h][: D + 1, hsl],
                        vn_h(kt_i // 4, kt_i % 4, h),
                        p_sb[:, hsl],
                        start=(kt_i == 0), stop=(kt_i == att_nt - 1),
                    )

            def emit_finish_stripe(m, accs):
                # copy both accumulators out of PSUM first so their banks
                # free for the next stripe while normalization runs on SBUF
                acc_sbs = []
                for h in range(HPC):
                    acc_sb = sb.tile([D + 1, QCHUNK], DTM, name="acc_sb", tag="acc_sb", bufs=2)
                    nc.vector.tensor_copy(acc_sb[:], accs[h][: D + 1, :])
                    acc_sbs.append(acc_sb)
                for h in range(HPC):
                    hs = slice(h * D, (h + 1) * D)
                    acc_sb = acc_sbs[h]
                    recip = sb.tile([1, QCHUNK], DTM, name="recip", tag="recip", bufs=2)
                    with nc.allow_low_precision(reason="softmax denom in bf16; tol 2e-2"):
                        nc.vector.reciprocal(recip[:], acc_sb[D : D + 1, :])
                    bcast = sb.tile([D, QCHUNK], DTM, name="bcast", tag="bcast", bufs=2)
                    nc.gpsimd.partition_broadcast(bcast[:], recip[:1, :])
                    nc.vector.tensor_mul(att_m[m][hs, :], acc_sb[:D, :], bcast[:])
                nc.sync.dma_start(
                    a2a_in[m][:].rearrange("a p t -> p a t"),
                    att_m[m][:].rearrange("p (a t) -> p a t", a=N_CORES),
                )
                if not skip_a2a:
                    nc.gpsimd.collective_compute(
                        "AllToAll",
                        mybir.AluOpType.bypass,
                        replica_groups=[list(range(N_CORES))],
                        ins=[a2a_in[m].opt()],
                        outs=[a2a_out[m].opt()],
                    )

            def emit_aTm_load(m):
                aTm = sb.tile([128, N_CORES * 128], DTM, name="aTm", tag="aTm", bufs=2)
                nc.sync.dma_start(
                    aTm[:].rearrange("p (a t) -> p a t", a=N_CORES),
                    a2a_out[m][:].rearrange("a p t -> p a t"),
                )
                return aTm

            def outproj_pieces(m, aTm_ref, cc, npiece=4):
                """output projection for stripe m, 512-column half cc, as
                npiece thunks accumulating into an SBUF tile via DVE."""
                os_ = slice(cc * 512, (cc + 1) * 512)
                per = HT // npiece
                holder = []
                thunks = []
                for pc in range(npiece):
                    def piece(pc=pc):
                        po = ps.tile([128, 512], DT, name="po", tag="ps_big")
                        for i in range(per * pc, per * (pc + 1)):
                            nc.tensor.matmul(
                                po[:], aTm_ref[0][:, i * 128 : (i + 1) * 128],
                                wo_sb[i][:, os_],
                                start=(i == per * pc), stop=(i == per * (pc + 1) - 1),
                            )
                        if pc == 0:
                            out_sb = sb.tile([128, 512], DT, name="out_sb", tag="out_sb", bufs=2)
                            holder.append(out_sb)
                            nc.vector.tensor_add(out_sb[:], po[:], bo_bc[:, os_])
                        else:
                            out_sb = holder[0]
                            nc.vector.tensor_add(out_sb[:], out_sb[:], po[:])
                        if pc == npiece - 1:
                            nc.sync.dma_start(out[m * 128 : (m + 1) * 128, os_], out_sb[:])
                    thunks.append(piece)
                return thunks

            # ---- schedule ----------------------------------------------
            def new_accs():
                return [
                    ps.tile([128, QCHUNK], DT, name=f"acc{h}", tag="ps_acc")
                    for h in range(HPC)
                ]

            # phase 1: stream chunks with stripe-0 attention interleaved at
            # lag 1 (chunk cp delivers k-tiles 4cp..4cp+3; attention trails
            # one chunk behind so exp work reaches ACT as early as possible).
            # Phase 1 is PE/supply-bound, so projection blocks sit between
            # attention groups without extra cost.
            # chunk-1's q runs before chunk-0's v so the first scores+exp
            # fire as early as possible
            emit_weight_loads()
            emit_xt(0)
            emit_xt(1)
            for t in k_pieces(0) + q_pieces(0) + q_pieces(1) + v_pieces(0):
                t()
            # out-proj weights load early on the SP queue (x loads are on
            # Pool, so these only queue behind qkv weights)
            for i in range(HT):
                nc.sync.dma_start(wo_sb[i][:], wo[i * 128 : (i + 1) * 128, :])
            bo_sb = sb.tile([1, HIDDEN], DT)
            nc.sync.dma_start(bo_sb[:], bo[:])
            nc.gpsimd.partition_broadcast(bo_bc[:], bo_sb[:1, :])

            accs = new_accs()
            for cp in range(1, HT):
                if cp > 1:
                    emit_xt(cp)
                    pieces = k_pieces(cp) + q_pieces(cp) + v_pieces(cp)
                else:
                    pieces = k_pieces(cp) + v_pieces(cp)
                a = 4 * (cp - 1)
                for kk in range(4):
                    for h in range(HPC):
                        emit_att_kt(0, a + kk, h, accs)
                    if kk == 1:
                        for t in pieces[:3]:
                            t()
                    elif kk == 3:
                        for t in pieces[3:]:
                            t()
            for kt_i in range(4 * (HT - 1), att_nt):
                for h in range(HPC):
                    emit_att_kt(0, kt_i, h, accs)
            emit_finish_stripe(0, accs)

            # stripes 1..3: previous stripe's out-projection rides inside
            # the ACT-bound kt loop as two single-block injection sites
            for m in range(1, NQC):
                accs = new_accs()
                aTm_ref = []
                cc0 = outproj_pieces(m - 1, aTm_ref, 0, npiece=1)
                cc1 = outproj_pieces(m - 1, aTm_ref, 1, npiece=1)
                slots = {16: cc0, 24: cc1}
                for kt_i in range(att_nt):
                    if kt_i == 8:
                        aTm_ref.append(emit_aTm_load(m - 1))
                    for h in range(HPC):
                        emit_att_kt(m, kt_i, h, accs)
                    for t in slots.get(kt_i, ()):
                        t()
                emit_finish_stripe(m, accs)
            aTm_ref = [emit_aTm_load(NQC - 1)]
            for cc in range(2):
                for t in outproj_pieces(NQC - 1, aTm_ref, cc, npiece=1):
                    t()

    nc.compile()
    return nc


def _get_nc(mm_mode: str):
    if mm_mode not in _CACHE:
        _CACHE[mm_mode] = _build(mm_mode)
    return _CACHE[mm_mode]


def make_in_maps(x, w_qkv, b_qkv, w_out, b_out):
    import ml_dtypes

    bf16 = ml_dtypes.bfloat16
    x = np.asarray(x, dtype=np.float32)
    w_qkv = np.asarray(w_qkv, dtype=np.float32)
    b_qkv = np.asarray(b_qkv, dtype=np.float32)
    w_out = np.asarray(w_out, dtype=np.float32)
    b_out = np.asarray(b_out, dtype=np.float32)

    xT = x.reshape(N, HIDDEN).T  # [hidden, n]
    # permute n into stripe order n' = (m, j, t) <-> n = 512*j + 128*m + t
    xT = np.ascontiguousarray(
        xT.reshape(HIDDEN, N_CORES, NQC, 128).transpose(0, 2, 1, 3).reshape(HIDDEN, N)
    ).astype(bf16)
    w_out_bf = np.ascontiguousarray(w_out).astype(bf16)
    bo = np.ascontiguousarray(b_out.reshape(1, HIDDEN))
    in_maps = []
    for c in range(N_CORES):
        cs = slice(c * AD, (c + 1) * AD)
        in_maps.append(
            {
                "xT": xT,
                "wq": np.ascontiguousarray(w_qkv[:, :HIDDEN][:, cs]).astype(bf16),
                "wk": np.ascontiguousarray(w_qkv[:, HIDDEN : 2 * HIDDEN][:, cs]).astype(bf16),
                "wv": np.ascontiguousarray(w_qkv[:, 2 * HIDDEN :][:, cs]).astype(bf16),
                "bq": np.ascontiguousarray(b_qkv[:HIDDEN][cs].reshape(AD, 1)),
                "bk": np.ascontiguousarray(b_qkv[HIDDEN : 2 * HIDDEN][cs].reshape(AD, 1)),
                "bvT": np.ascontiguousarray(b_qkv[2 * HIDDEN :][cs].reshape(1, AD)),
                "wo": w_out_bf,
                "bo": bo,
            }
        )
    return in_maps


def kernel(x, w_qkv, b_qkv, w_out, b_out):
    from concourse.bass_utils import run_bass_kernel_spmd

    mm_mode = os.environ.get("TRN_MM_MODE", "bf16")
    nc = _get_nc(mm_mode)
    in_maps = make_in_maps(x, w_qkv, b_qkv, w_out, b_out)
    res = run_bass_kernel_spmd(nc, in_maps, list(range(N_CORES)))
    full = np.concatenate([res.results[c]["out"] for c in range(N_CORES)], axis=0)
    return full.reshape(1, N, HIDDEN).astype(np.float32)
